# revision 22
# baseline (speedup 1.0000x reference)
"""Trainium2 Bass kernel for nn_LlamaAttention_6588479832091.

Math notes:
  - The reference attention contracts q and k at the SAME sequence position
    (scores = einsum('bshd,bstd->bsht', q, k)), and RoPE applies the same
    orthogonal transform to q and k at equal positions, so RoPE cancels
    exactly: (P R q)·(P R k) = q·k.  v and the output path never see RoPE.
    The kernel computes: q/k/v projections, per-token 16x16 cross-head
    softmax attention, and the output projection.
  - Sharding: data-parallel over the 16384 tokens -> 2048 tokens per core,
    weights replicated.  No collectives.
  - Everything on the PE runs in bf16 (1 cycle/row at any output width, vs
    4 for fp32 / 1.5-4 for f32r at narrow widths).  PSUM accumulates fp32.
    Measured end-to-end rel err ~5.5e-3, well inside the 2e-2 gate.
  - Fused single pass over 512-token chunks: QKV psums evacuate DIRECTLY
    into the packed cross-head attention layout in SBUF (no DRAM roundtrip
    for q/k/v, no repack).  Chunk c's attention/softmax pipeline is emitted
    INTERLEAVED between chunk c+1's projection chains so the in-order PE
    queue always has matmul work while DVE/ACT/Pool run the softmax -- the
    attention phase costs ~zero wall-clock.
  - Softmax denominator via gpsimd partition_all_reduce (result broadcast
    over partitions), 1/z applied to the exp'd scores BEFORE the swapped
    MM2 (v^T stationary), so the MM2 output is already attn^T in [d, h, t]
    layout -- no re-transpose and no per-output normalize needed.

Layouts (host-prepared; all contiguous per-partition slabs):
  xt   [128, 4, 16, 512]  xt[p, c, ct, t] = x_shard[c*512+t, ct*128+p] (bf16)
  wqt  [128, 16, 16, 128] wqt[p, mt, ct, j] = wq[mt*128+j, ct*128+p]/sqrt(128)
  wkt, wvt, wot: same tiling as wqt (unscaled)                        (bf16)
  mask [128, 512]  0 where p%8 == n%8 else -30000, tiled x4           (f32)
  ident[128, 128]  identity                                           (bf16)
  ot   [128, 16, 2048]  ot[p, rt, t] = out_shard[t, rt*128+p]         (f32, out)

Attention pack layout per chunk (512 tokens = 64 groups of GRP=8):
  qpk[d, g, h*8+tj] = q[d, head h, token g*8+tj]   (d = head dim 0..127)
  MM1 per group: scores[(hk,tk),(hq,tj)] = k_g^T @ q_g; the mask kills
  tk != tj so softmax runs over the 16 heads at the same token.
"""

import os
import sys

os.environ.setdefault("NEURON_RT_RESET_CORES", "1")

for _p in ("/opt/trn_rl_repo", "/root/.axon_site/_ro/trn_rl_repo"):
    if _p not in sys.path:
        sys.path.insert(0, _p)

import numpy as np

T_CORE = 2048      # tokens per core
N_CORES = 8
H = 16             # heads
HD = 128           # head dim
HIDDEN = 2048
CT = HIDDEN // 128  # 16 contraction tiles
TCH = 512          # token chunk (fused proj+attn+o-proj granularity)
SUB = 128          # attention sub-chunk tokens
GRP = 8            # tokens per attention group

_CACHED = {}


def _build(phases="PAO"):
    import concourse.mybir as mybir
    import concourse.tile as tile
    import concourse.bacc as bacc
    from concourse.bass_isa import ReduceOp

    f32 = mybir.dt.float32
    bf16 = mybir.dt.bfloat16

    nc = bacc.Bacc("TRN2", target_bir_lowering=False, debug=False)

    NCH_ = T_CORE // TCH
    xt = nc.declare_dram_parameter("xt", [128, NCH_, CT, TCH], bf16, isOutput=False)
    wqt = nc.declare_dram_parameter("wqt", [128, H, CT, 128], bf16, isOutput=False)
    wkt = nc.declare_dram_parameter("wkt", [128, H, CT, 128], bf16, isOutput=False)
    wvt = nc.declare_dram_parameter("wvt", [128, H, CT, 128], bf16, isOutput=False)
    wot = nc.declare_dram_parameter("wot", [128, H, CT, 128], bf16, isOutput=False)
    maskd = nc.declare_dram_parameter("maskd", [128, 512], f32, isOutput=False)
    identd = nc.declare_dram_parameter("identd", [128, 128], bf16, isOutput=False)
    ot = nc.declare_dram_parameter("ot", [128, CT, T_CORE], f32, isOutput=True)

    NCH = T_CORE // TCH          # 4 chunks
    NGC = TCH // GRP             # 64 groups per chunk
    NSUB = TCH // SUB            # 4 subs per chunk
    NG = SUB // GRP              # 16 groups per sub
    NM = NG // 4                 # 4 macros per sub
    MAC = 4 * GRP                # 32 tokens per macro

    with tile.TileContext(nc) as tc:
        with tc.tile_pool(name="io", bufs=1) as io, \
             tc.tile_pool(name="wpool", bufs=6) as wpool, \
             tc.tile_pool(name="wk", bufs=3) as wk, \
             tc.tile_pool(name="ps", bufs=1, space="PSUM") as ps:
            mask_sb = io.tile([128, 512], f32, tag="mask")
            ident_sb = io.tile([128, 128], bf16, tag="ident")
            nc.scalar.dma_start(mask_sb[:], maskd[:])
            nc.scalar.dma_start(ident_sb[:], identd[:])


            x_tiles = [None] * NCH

            def emit_x_load(c):
                quarters = []
                for qi in range(4):
                    xq = io.tile([128, 4, TCH], bf16, tag="x", bufs=5,
                                 name=f"x{c}_{qi}")
                    nc.gpsimd.dma_start(
                        xq[:], xt[:, c, qi * 4:(qi + 1) * 4, :])
                    quarters.append(xq)
                x_tiles[c] = quarters

            emit_x_load(0)

            def stage1(st, m):
                """MM1 x4 + mask + exp for macro m."""
                ps_s = ps.tile([128, 512], f32, tag="s", bufs=3)
                for i in range(4):
                    g = 4 * m + i
                    nc.tensor.matmul(ps_s[:, i * 128:(i + 1) * 128],
                                     st["kpk"][:, g, :], st["qpk"][:, g, :],
                                     start=True, stop=True)
                nc.vector.tensor_add(ps_s[:], ps_s[:], mask_sb[:])
                wt = wk.tile([128, 512], bf16, tag="wt_sb", bufs=5)
                nc.scalar.activation(wt[:], ps_s[:],
                                     mybir.ActivationFunctionType.Exp)
                st[("wt", m)] = wt

            def stage2(st, m):
                """z all-reduce + 1/z + V-transpose + vp evac for macro m."""
                wt = st[("wt", m)]
                zbc = wk.tile([128, 512], f32, tag="zbc", bufs=2)
                nc.gpsimd.partition_all_reduce(zbc[:], wt[:], channels=128,
                                               reduce_op=ReduceOp.add)
                nc.vector.reciprocal(zbc[:], zbc[:])
                st[("zbc", m)] = zbc
                ps_v = ps.tile([128, 512], bf16, tag="vp", bufs=1)
                for i in range(4):
                    g = 4 * m + i
                    nc.tensor.transpose(ps_v[:, i * 128:(i + 1) * 128],
                                        st["vpk"][:, g, :], ident_sb[:])
                vp = wk.tile([128, 512], bf16, tag="vp_sb", bufs=4)
                nc.vector.tensor_copy(vp[:], ps_v[:])
                st[("vp", m)] = vp

            def stage3(st, m):
                """Normalize wt by 1/z for macro m."""
                wt = st.pop(("wt", m))
                zbc = st.pop(("zbc", m))
                wtn = wk.tile([128, 512], bf16, tag="wtn", bufs=4)
                nc.vector.tensor_mul(wtn[:], wt[:], zbc[:])
                st[("wtn", m)] = wtn

            def stage4(st, m):
                """Swapped MM2 (attn^T = vp^T @ wtn) + at-copy for macro m."""
                vp = st.pop(("vp", m))
                wtn = st.pop(("wtn", m))
                ps_o = ps.tile([128, 512], f32, tag="attn", bufs=1)
                for i in range(4):
                    nc.tensor.matmul(ps_o[:, i * 128:(i + 1) * 128],
                                     vp[:, i * 128:(i + 1) * 128],
                                     wtn[:, i * 128:(i + 1) * 128],
                                     start=True, stop=True)
                nc.scalar.copy(
                    st["at"][:, :, m * MAC:(m + 1) * MAC].rearrange(
                        "p h (g ti) -> p g h ti", g=4),
                    ps_o[:].rearrange("p (g h ti) -> p g h ti", g=4, h=H))

            NMC = NGC // 4           # 16 macros per chunk
            NSTEP = NMC + 5          # attention pipeline steps per chunk

            def make_ctx(c):
                return {
                    "qpk": io.tile([128, NGC, 128], bf16, tag="qpk", bufs=2,
                                   name=f"qpk{c}"),
                    "kpk": io.tile([128, NGC, 128], bf16, tag="kpk", bufs=2,
                                   name=f"kpk{c}"),
                    "vpk": io.tile([128, NGC, 128], bf16, tag="vpk", bufs=2,
                                   name=f"vpk{c}"),
                    "at": io.tile([128, H, TCH], bf16, tag="at", bufs=2,
                                  name=f"at{c}"),
                }

            def attn_steps(st):
                """One yield per software-pipeline step of a chunk's attention."""
                for M in range(NSTEP):
                    if M < NMC:
                        stage1(st, M)
                    if 2 <= M < NMC + 2:
                        stage2(st, M - 2)
                    if 3 <= M < NMC + 3:
                        stage3(st, M - 3)
                    if 5 <= M < NMC + 5:
                        stage4(st, M - 5)
                    yield

            def emit_proj(c, ctx, interleave=None):
                x_sb = x_tiles[c]
                chains = 3 * H
                done = [0]

                def pace(chain_idx):
                    if interleave is None:
                        return
                    while done[0] * chains < chain_idx * NSTEP:
                        if next(interleave, "END") == "END":
                            break
                        done[0] += 1

                ci = 0
                for wsrc, key in ((wqt, "qpk"), (wkt, "kpk"), (wvt, "vpk")):
                    for mt in range(H):
                        wslab = wpool.tile([128, CT, 128], bf16, tag="wslab",
                                           bufs=5)
                        nc.sync.dma_start(wslab[:], wsrc[:, mt, :, :])
                        psum = ps.tile([128, TCH], f32, tag="pp", bufs=3)
                        for kt in range(CT):
                            nc.tensor.matmul(
                                psum[:],
                                wslab[:, kt, :],
                                x_sb[kt // 4][:, kt % 4, :],
                                start=(kt == 0), stop=(kt == CT - 1))
                        nc.vector.tensor_copy(
                            ctx[key][:, :, mt * GRP:(mt + 1) * GRP],
                            psum[:].rearrange("p (g tj) -> p g tj", tj=GRP))
                        ci += 1
                        pace(ci)
                if interleave is not None:
                    for _ in interleave:
                        pass

            def emit_oproj(c, ctx, interleave=None):
                done = [0]

                def pace(chain_idx):
                    if interleave is None:
                        return
                    while done[0] * CT < chain_idx * NSTEP:
                        if next(interleave, "END") == "END":
                            break
                        done[0] += 1

                for rt in range(CT):
                    woslab = wpool.tile([128, CT, 128], bf16, tag="woslab",
                                        bufs=3, name="woslab")
                    nc.sync.dma_start(woslab[:], wot[:, rt, :, :])
                    po = ps.tile([128, TCH], f32, tag="pp", bufs=3)
                    for kt in range(CT):
                        nc.tensor.matmul(
                            po[:],
                            woslab[:, kt, :],
                            ctx["at"][:, kt, :],
                            start=(kt == 0), stop=(kt == CT - 1))
                    oev = io.tile([128, TCH], f32, tag="oev", bufs=3,
                                  name="oev")
                    nc.vector.tensor_copy(oev[:], po[:])
                    nc.scalar.dma_start(
                        ot[:, rt, c * TCH:(c + 1) * TCH], oev[:])
                    pace(rt + 1)

            ctxs = {}
            ctxs[0] = make_ctx(0)
            emit_proj(0, ctxs[0])
            emit_x_load(1)
            for c in range(1, NCH):
                ctxs[c] = make_ctx(c)
                emit_proj(c, ctxs[c], attn_steps(ctxs[c - 1]))
                if c + 1 < NCH:
                    emit_x_load(c + 1)
                if c < NCH - 1:
                    emit_oproj(c - 1, ctxs[c - 1])
                else:
                    emit_oproj(c - 1, ctxs[c - 1], attn_steps(ctxs[c]))
                    emit_oproj(c, ctxs[c])
    nc.compile()
    return nc


def _host_prep(x, wq, wk, wv, wo):
    """Build per-core input maps (layout transforms only)."""
    import ml_dtypes
    bf16 = ml_dtypes.bfloat16

    x2 = np.ascontiguousarray(x.reshape(-1, HIDDEN))          # (16384, 2048)
    wqs = (wq / np.sqrt(np.float32(HD))).astype(np.float32)

    def wt3(w):   # (m, c) weight -> [128, H, CT, 128]: [p, mt, ct, col]
        wt = np.ascontiguousarray(w.T)                        # (c, m)
        return np.ascontiguousarray(
            wt.reshape(CT, 128, H, 128).transpose(1, 2, 0, 3)).astype(bf16)

    wqt, wkt, wvt, wot = wt3(wqs), wt3(wk), wt3(wv), wt3(wo)
    p = np.arange(128)[:, None]
    n = np.arange(128)[None, :]
    mask = np.where((p % GRP) == (n % GRP), 0.0, -30000.0).astype(np.float32)
    mask = np.tile(mask, (1, 4))
    ident = np.eye(128, dtype=np.float32).astype(bf16)

    in_maps = []
    for c in range(N_CORES):
        xs = x2[c * T_CORE:(c + 1) * T_CORE]                  # (2048, 2048)
        xtile = np.ascontiguousarray(
            xs.T.reshape(CT, 128, T_CORE // TCH, TCH)
            .transpose(1, 2, 0, 3)).astype(bf16)
        in_maps.append({"xt": xtile, "wqt": wqt, "wkt": wkt, "wvt": wvt,
                        "wot": wot, "maskd": mask, "identd": ident})
    return in_maps


def kernel(x, wq, wk, wv, wo, inv_freq):
    # inv_freq is unused: RoPE is an identical orthogonal transform on q and k
    # at equal positions, and this attention only contracts same-position q·k,
    # so it cancels exactly.
    from concourse.bass_utils import run_bass_kernel_spmd

    x = np.asarray(x, dtype=np.float32)
    wq = np.asarray(wq, dtype=np.float32)
    wk = np.asarray(wk, dtype=np.float32)
    wv = np.asarray(wv, dtype=np.float32)
    wo = np.asarray(wo, dtype=np.float32)

    if "nc" not in _CACHED:
        _CACHED["nc"] = _build()
    nc = _CACHED["nc"]

    in_maps = _host_prep(x, wq, wk, wv, wo)
    res = run_bass_kernel_spmd(nc, in_maps, core_ids=list(range(N_CORES)))

    out = np.empty((N_CORES * T_CORE, HIDDEN), dtype=np.float32)
    for c in range(N_CORES):
        otc = res.results[c]["ot"]                             # (128, 16, 2048)
        out[c * T_CORE:(c + 1) * T_CORE] = (
            otc.transpose(2, 1, 0).reshape(T_CORE, HIDDEN))
    return out.reshape(x.shape[0], x.shape[1], HIDDEN)
